# revision 15
# baseline (speedup 1.0000x reference)
"""Trainium2 Bass kernel for nn_BioSimulator (phosphene pooling model).

Math: the reference materializes dist2/gauss of shape (1, 1024, 256, 256) and
reduces over the 1024 electrodes.  dist2 is separable:
    dist2[n,h,w] = ((px[w]-vx[n])*s)^2 + ((py[h]-vy[n])*s)^2
so   gauss[n,h,w] = gx[n,w] * gy[n,h]   with
    gx[n,w] = exp(-((px[w]-vx[n])*s)^2 / (2 sigma_n^2))
    gy[n,h] = exp(-((py[h]-vy[n])*s)^2 / (2 sigma_n^2))
and  out[h,w]  = sum_n Bamp[n] * gy[n,h] * gx[n,w]  — a (H x N) @ (N x W)
matmul with K = 1024.  The per-electrode model configuration (wedge-dipole
retinotopy via complex exp/div, sigma, Bamp) is computed on-chip on [128, 8]
tiles (electrode n = 128*j + p: partition p, chunk column j).

Sharding: 2x4 grid over the output — core c computes the h-half hh = c // 4
(128 rows) and w-quarter wq = c % 4 (64 cols).  Every core evaluates all 1024
electrodes for its slice (fully local, no collectives); the hosts stitches the
8 [128, 64] slices into the (1, 1, 256, 256) output.
"""

import numpy as np

GRID = 32
OUT = 256
FOV = 30.0
N_CORES = 8
NCHUNK = 8  # 1024 electrodes / 128 partitions

K_, A_, B_ = 17.3, 0.75, 120.0
SLOPE, HALF, RHEO = 19152642.5, 1.057e-07, 2.39e-05
FREQ, PW, R2S = 300.0, 0.00017, 0.5
DEG2PIX = OUT / (2.0 * FOV)
HALF_PI = float(np.pi / 2.0)
DEG2RAD = float(np.pi / 180.0)

_CACHE: dict = {}


def _host_constants():
    """Electrode / pixel grids (input-independent)."""
    if "consts" in _CACHE:
        return _CACHE["consts"]
    xc = np.linspace(-15.0, 15.0, GRID, dtype=np.float32)
    gx, gy = np.meshgrid(xc, xc, indexing="xy")
    # electrode n = 128*j + p  ->  [128, 8] with [p, j] = flat[j*128 + p]
    gxe = gx.reshape(-1).astype(np.float32).reshape(NCHUNK, 128).T.copy()
    gye = gy.reshape(-1).astype(np.float32).reshape(NCHUNK, 128).T.copy()
    xs = np.linspace(-FOV, FOV, OUT, dtype=np.float32)
    _CACHE["consts"] = (gxe, gye, xs)
    return _CACHE["consts"]


def _build_nc():
    """Build the SPMD Bass/Tile program (same program on all 8 cores)."""
    if "nc" in _CACHE:
        return _CACHE["nc"]

    import concourse.bacc as bacc
    import concourse.mybir as mybir
    import concourse.tile as tile

    f32 = mybir.dt.float32
    AF = mybir.ActivationFunctionType
    OP = mybir.AluOpType

    # Bacc (not raw Bass): its compile pipeline runs generate_event_semaphores,
    # which splits multi-sem waits — TRN2 instructions carry at most one wait.
    nc = bacc.Bacc(None)
    d_stim = nc.declare_dram_parameter("stim", [128, NCHUNK], f32, isOutput=False)
    d_pp = nc.declare_dram_parameter("pp", [128, 13], f32, isOutput=False)
    d_gxe = nc.declare_dram_parameter("gxe", [128, NCHUNK], f32, isOutput=False)
    d_gye = nc.declare_dram_parameter("gye", [128, NCHUNK], f32, isOutput=False)
    d_pxb = nc.declare_dram_parameter("pxb", [128, 64], f32, isOutput=False)
    d_pyb = nc.declare_dram_parameter("pyb", [128, 128], f32, isOutput=False)
    d_o = nc.declare_dram_parameter("o", [128, 64], f32, isOutput=True)

    with tile.TileContext(nc) as tc:
        with (
            tc.tile_pool(name="cst", bufs=1) as cp,
            tc.tile_pool(name="prm", bufs=1) as pr,
            tc.tile_pool(name="wrk", bufs=9) as wp,
            tc.tile_pool(name="acc", bufs=1, space="PSUM") as ps,
        ):
            def cst(dram, w, tag):
                t = cp.tile([128, w], f32, tag=tag, name=tag)
                nc.sync.dma_start(t[:], dram[:])
                return t

            stim = cst(d_stim, NCHUNK, "stim")
            pp = cst(d_pp, 13, "pp")
            gxe = cst(d_gxe, NCHUNK, "gxe")
            gye = cst(d_gye, NCHUNK, "gye")
            pxb = cst(d_pxb, 64, "pxb")
            pyb = cst(d_pyb, 128, "pyb")

            def pt(tag, w=NCHUNK):
                return pr.tile([128, w], f32, tag=tag, name=tag)

            act = nc.scalar.activation
            tt = nc.vector.tensor_tensor
            ts = nc.vector.tensor_scalar

            # [128,1] constants for non-Copy activation biases (only 0/1 are
            # pre-registered in the const-AP database)
            c_halfpi = pt("c_halfpi", 1)
            nc.vector.memset(c_halfpi[:], HALF_PI)
            c_nsh = pt("c_nsh", 1)
            nc.vector.memset(c_nsh[:], -SLOPE * HALF)

            # pixel grids pre-scaled by deg2pix on DVE, so the per-chunk ACT
            # Square ops depend on a single (DVE) semaphore — the walrus AC
            # struct rejects instructions with more than one sync wait
            pxs = pt("pxs", 64)
            ts(pxs[:], pxb[:], DEG2PIX, None, OP.mult)
            pys = pt("pys", 128)
            ts(pys[:], pyb[:], DEG2PIX, None, OP.mult)

            # ---- per-patient scalars, broadcast on partitions: [128, 1] ----
            th = pt("th", 1)
            act(th[:], pp[:, 12:13], AF.Copy, scale=DEG2RAD)
            ct = pt("ct", 1)
            act(ct[:], th[:], AF.Sin, bias=c_halfpi[:, 0:1])
            st = pt("st", 1)
            act(st[:], th[:], AF.Sin)
            dxs = pt("dxs", 1)
            act(dxs[:], pp[:, 10:11], AF.Copy, scale=1.0 / 300.0)
            dys = pt("dys", 1)
            act(dys[:], pp[:, 11:12], AF.Copy, scale=1.0 / 300.0)
            rho9 = pt("rho9", 1)
            act(rho9[:], pp[:, 0:1], AF.Copy, bias=1e-09)
            irho = pt("irho", 1)
            nc.vector.reciprocal(irho[:], rho9[:])

            # ---- electrode transforms: [128, 8] ----
            t1 = pt("t1")
            ts(t1[:], gxe[:], ct[:, 0:1], None, OP.mult)
            t2 = pt("t2")
            ts(t2[:], gye[:], st[:, 0:1], None, OP.mult)
            gxn = pt("gxn")
            tt(gxn[:], t1[:], t2[:], OP.subtract)
            ts(gxn[:], gxn[:], dxs[:, 0:1], None, OP.add)
            t3 = pt("t3")
            ts(t3[:], gxe[:], st[:, 0:1], None, OP.mult)
            t4 = pt("t4")
            ts(t4[:], gye[:], ct[:, 0:1], None, OP.mult)
            gyn = pt("gyn")
            tt(gyn[:], t3[:], t4[:], OP.add)
            ts(gyn[:], gyn[:], dys[:, 0:1], None, OP.add)

            # exp((gxn + i*gyn)/k) = er * (cos + i sin)
            er = pt("er")
            act(er[:], gxn[:], AF.Exp, scale=1.0 / K_)
            co = pt("co")
            act(co[:], gyn[:], AF.Sin, scale=1.0 / K_, bias=c_halfpi[:, 0:1])
            si = pt("si")
            act(si[:], gyn[:], AF.Sin, scale=1.0 / K_)
            ewr = pt("ewr")
            tt(ewr[:], er[:], co[:], OP.mult)
            ewi = pt("ewi")
            tt(ewi[:], er[:], si[:], OP.mult)

            # z = a*b*(ew - 1) / (b - a*ew)   (complex)
            ab = A_ * B_
            nr = pt("nr")
            ts(nr[:], ewr[:], ab, -ab, OP.mult, OP.add)
            ni = pt("ni")
            ts(ni[:], ewi[:], ab, None, OP.mult)
            dr = pt("dr")
            ts(dr[:], ewr[:], -A_, B_, OP.mult, OP.add)
            di = pt("di")
            ts(di[:], ewi[:], -A_, None, OP.mult)
            den = pt("den")
            tt(den[:], dr[:], dr[:], OP.mult)
            t5 = pt("t5")
            tt(t5[:], di[:], di[:], OP.mult)
            tt(den[:], den[:], t5[:], OP.add)
            iden = pt("iden")
            nc.vector.reciprocal(iden[:], den[:])
            q1 = pt("q1")
            tt(q1[:], nr[:], dr[:], OP.mult)
            q2 = pt("q2")
            tt(q2[:], ni[:], di[:], OP.mult)
            tt(q1[:], q1[:], q2[:], OP.add)
            zr = pt("zr")
            tt(zr[:], q1[:], iden[:], OP.mult)
            q3 = pt("q3")
            tt(q3[:], ni[:], dr[:], OP.mult)
            q4 = pt("q4")
            tt(q4[:], nr[:], di[:], OP.mult)
            tt(q3[:], q3[:], q4[:], OP.subtract)
            zi = pt("zi")
            tt(zi[:], q3[:], iden[:], OP.mult)

            # r = |z|;  M = k*(1/(r+a) - 1/(r+b))
            r2 = pt("r2")
            tt(r2[:], zr[:], zr[:], OP.mult)
            t6 = pt("t6")
            tt(t6[:], zi[:], zi[:], OP.mult)
            tt(r2[:], r2[:], t6[:], OP.add)
            rr = pt("rr")
            act(rr[:], r2[:], AF.Sqrt)
            rpa = pt("rpa")
            ts(rpa[:], rr[:], A_, None, OP.add)
            ira = pt("ira")
            nc.vector.reciprocal(ira[:], rpa[:])
            rpb = pt("rpb")
            ts(rpb[:], rr[:], B_, None, OP.add)
            irb = pt("irb")
            nc.vector.reciprocal(irb[:], rpb[:])
            mk = pt("mk")
            tt(mk[:], ira[:], irb[:], OP.subtract)
            me = pt("me")
            ts(me[:], mk[:], K_, 1e-09, OP.mult, OP.add)  # M + 1e-9
            uu = pt("uu")
            nc.vector.reciprocal(uu[:], me[:])

            # sigma_px = max(sqrt(I/(rho+1e-9)) * r2s/(M+1e-9) * deg2pix, 0.5)
            sb2 = pt("sb2")
            ts(sb2[:], stim[:], irho[:, 0:1], None, OP.mult)
            sb = pt("sb")
            act(sb[:], sb2[:], AF.Sqrt, scale=8e-05)
            vv = pt("vv")
            tt(vv[:], sb[:], uu[:], OP.mult)
            sg = pt("sg")
            ts(sg[:], vv[:], R2S * DEG2PIX, 0.5, OP.mult, OP.max)
            # rs = 1/(sqrt(2)*sigma), folded into the Square's input so the
            # following Exp uses an immediate scale of -1 (single-wait ACT)
            rsd = pt("rsd")
            ts(rsd[:], sg[:], float(np.sqrt(2.0)), None, OP.mult)
            rs = pt("rs")
            nc.vector.reciprocal(rs[:], rsd[:])

            # Bamp = sigmoid(slope*(relu(I - rheo)*pw*freq - half));  lnb = ln(Bamp)
            tie = pt("tie")
            ts(tie[:], stim[:], 8e-05, -RHEO, OP.mult, OP.add)
            ie = pt("ie")
            act(ie[:], tie[:], AF.Relu)
            bamp = pt("bamp")
            act(
                bamp[:],
                ie[:],
                AF.Sigmoid,
                scale=SLOPE * PW * FREQ,
                bias=c_nsh[:, 0:1],
            )
            lnb = pt("lnb")
            act(lnb[:], bamp[:], AF.Ln)

            # negated centers, scaled by deg2pix * rs, for the Square bias
            nvx = pt("nvx")
            ts(nvx[:], zr[:], -DEG2PIX, None, OP.mult)
            tt(nvx[:], nvx[:], rs[:], OP.mult)
            nvy = pt("nvy")
            ts(nvy[:], zi[:], -DEG2PIX, None, OP.mult)
            tt(nvy[:], nvy[:], rs[:], OP.mult)

            # ---- main loop: 8 electrode chunks -> accumulate matmul ----
            acc = ps.tile([128, 64], f32, tag="acc", name="acc")
            for j in range(NCHUNK):
                jc = slice(j, j + 1)
                sqx = wp.tile([128, 64], f32, tag="sqx", name="sqx")
                act(sqx[:], pxs[:], AF.Square, scale=rs[:, jc], bias=nvx[:, jc])
                gxm = wp.tile([128, 64], f32, tag="gxm", name="gxm")
                act(gxm[:], sqx[:], AF.Exp, scale=-1.0)
                sqy = wp.tile([128, 128], f32, tag="sqy", name="sqy")
                act(sqy[:], pys[:], AF.Square, scale=rs[:, jc], bias=nvy[:, jc])
                gym = wp.tile([128, 128], f32, tag="gym", name="gym")
                act(gym[:], sqy[:], AF.Exp, scale=-1.0, bias=lnb[:, jc])
                nc.tensor.matmul(
                    acc[:], gym[:], gxm[:], start=(j == 0), stop=(j == NCHUNK - 1)
                )

            # ---- polynomial + clip on the [128, 64] slice ----
            a0, a1, a2, a3, a4 = (pp[:, 3 + i : 4 + i] for i in range(5))
            ot = wp.tile([128, 64], f32, tag="ot", name="ot")
            nc.vector.tensor_copy(ot[:], acc[:])
            po = wp.tile([128, 64], f32, tag="po", name="po")
            ts(po[:], ot[:], a4, a3, OP.mult, OP.add)
            tt(po[:], po[:], ot[:], OP.mult)
            ts(po[:], po[:], a2, None, OP.add)
            tt(po[:], po[:], ot[:], OP.mult)
            ts(po[:], po[:], a1, None, OP.add)
            tt(po[:], po[:], ot[:], OP.mult)
            ts(po[:], po[:], a0, 0.0, OP.add, OP.max)
            ts(po[:], po[:], 1.0, None, OP.min)
            nc.sync.dma_start(d_o[:], po[:])

    nc.finalize()
    _CACHE["nc"] = nc
    return nc


def _prep_in_maps(stim_np: np.ndarray, pp_np: np.ndarray):
    gxe, gye, xs = _host_constants()
    stim_dev = (
        stim_np.reshape(-1).astype(np.float32).reshape(NCHUNK, 128).T.copy()
    )
    pp_dev = np.ascontiguousarray(
        np.broadcast_to(pp_np.reshape(1, 13).astype(np.float32), (128, 13))
    )
    in_maps = []
    for c in range(N_CORES):
        hh, wq = c // 4, c % 4
        pxb = np.ascontiguousarray(
            np.broadcast_to(xs[64 * wq : 64 * wq + 64][None, :], (128, 64))
        )
        pyb = np.ascontiguousarray(
            np.broadcast_to(xs[128 * hh : 128 * hh + 128][None, :], (128, 128))
        )
        in_maps.append(
            {
                "stim": stim_dev,
                "pp": pp_dev,
                "gxe": gxe,
                "gye": gye,
                "pxb": pxb,
                "pyb": pyb,
            }
        )
    return in_maps


def _assemble(results) -> np.ndarray:
    out = np.empty((OUT, OUT), dtype=np.float32)
    for c in range(N_CORES):
        hh, wq = c // 4, c % 4
        out[128 * hh : 128 * hh + 128, 64 * wq : 64 * wq + 64] = results[c]["o"]
    return out.reshape(1, 1, OUT, OUT)


def kernel(stimulation: np.ndarray, patient_params: np.ndarray) -> np.ndarray:
    from concourse.bass_utils import run_bass_kernel_spmd

    stim_np = np.asarray(stimulation, dtype=np.float32)
    pp_np = np.asarray(patient_params, dtype=np.float32)
    nc = _build_nc()
    in_maps = _prep_in_maps(stim_np, pp_np)
    res = run_bass_kernel_spmd(nc, in_maps, list(range(N_CORES)))
    return _assemble(res.results)


# revision 17
# speedup vs baseline: 1.3162x; 1.3162x over previous
"""Trainium2 Bass kernel for nn_BioSimulator (phosphene pooling model).

Math: the reference materializes dist2/gauss of shape (1, 1024, 256, 256) and
reduces over the 1024 electrodes.  dist2 is separable:
    dist2[n,h,w] = ((px[w]-vx[n])*s)^2 + ((py[h]-vy[n])*s)^2
so   gauss[n,h,w] = gx[n,w] * gy[n,h]   with
    gx[n,w] = exp(-((px[w]-vx[n])*s*rs_n)^2),  rs_n = 1/(sqrt(2)*sigma_n)
and  out[h,w]  = sum_n Bamp[n] * gy[n,h] * gx[n,w]  — a (H x N) @ (N x W)
matmul with K = 1024.  The per-electrode model configuration (wedge-dipole
retinotopy via complex exp/div, sigma, Bamp) is computed on-chip on [128, 8]
tiles (electrode n = 128*j + p: partition p, chunk column j).

ACT-table discipline: the scalar engine reloads its lookup table (~1.3 us)
whenever the activation function leaves the loaded set, so this kernel only
uses EXP and LN (which share the natural_log_exp_and_others set) plus the
table-free SQUARE.  sin/cos are degree-9/8 polynomials on the vector engine,
sqrt(x) = exp(0.5*ln(x)), and sigmoid = 1/(1 + e^sh * exp(-slope*q)) via DVE
reciprocal.  One table load total.

Sharding: 2x4 grid over the output — core c computes the h-half hh = c // 4
(128 rows) and w-quarter wq = c % 4 (64 cols).  Every core evaluates all 1024
electrodes for its slice (fully local, no collectives); the host stitches the
8 [128, 64] slices into the (1, 1, 256, 256) output.
"""

import numpy as np

GRID = 32
OUT = 256
FOV = 30.0
N_CORES = 8
NCHUNK = 8  # 1024 electrodes / 128 partitions

K_, A_, B_ = 17.3, 0.75, 120.0
SLOPE, HALF, RHEO = 19152642.5, 1.057e-07, 2.39e-05
FREQ, PW, R2S = 300.0, 0.00017, 0.5
DEG2PIX = OUT / (2.0 * FOV)
DEG2RAD = float(np.pi / 180.0)
INVK = 1.0 / K_
AB = A_ * B_
SLP = SLOPE * PW * FREQ          # 976784.7675
ESH = float(np.exp(SLOPE * HALF))  # e^{slope*half}

# sin(x) = x * P(x^2), cos(x) = Q(x^2); least-squares fits on |x| <= 0.9,
# max abs error ~8e-8 in fp32 (used for the gyn/k rotation angle)
SIN_C = (0.9999999999882416, -0.1666666658678421, 0.008333324780098869,
         -0.00019838097971974124, 2.708056858978883e-06)
COS_C = (0.9999999998709687, -0.49999999123379646, 0.041666572790482734,
         -0.0013885406730890894, 2.427793810618373e-05)

# packed input column layout: [stim | pp | gxe | gye | pxs | pys]
C_STIM, C_PP, C_GXE, C_GYE, C_PXS, C_PYS, C_END = 0, 8, 21, 29, 37, 101, 229

_CACHE: dict = {}


def _host_constants():
    """Electrode / pixel grids (input-independent)."""
    if "consts" in _CACHE:
        return _CACHE["consts"]
    xc = np.linspace(-15.0, 15.0, GRID, dtype=np.float32)
    gx, gy = np.meshgrid(xc, xc, indexing="xy")
    # electrode n = 128*j + p  ->  [128, 8] with [p, j] = flat[j*128 + p]
    gxe = gx.reshape(-1).astype(np.float32).reshape(NCHUNK, 128).T.copy()
    gye = gy.reshape(-1).astype(np.float32).reshape(NCHUNK, 128).T.copy()
    xs = np.linspace(-FOV, FOV, OUT, dtype=np.float32)
    _CACHE["consts"] = (gxe, gye, xs)
    return _CACHE["consts"]


def _build_nc():
    """Build the SPMD Bass/Tile program (same program on all 8 cores)."""
    if "nc" in _CACHE:
        return _CACHE["nc"]

    import concourse.bacc as bacc
    import concourse.mybir as mybir
    import concourse.tile as tile

    f32 = mybir.dt.float32
    AF = mybir.ActivationFunctionType
    OP = mybir.AluOpType

    # Bacc (not raw Bass): its compile pipeline runs generate_event_semaphores,
    # which splits multi-sem waits — TRN2 instructions carry at most one wait.
    nc = bacc.Bacc(None)
    d_inp = nc.declare_dram_parameter("inp", [128, C_END], f32, isOutput=False)
    d_o = nc.declare_dram_parameter("o", [128, 64], f32, isOutput=True)

    with tile.TileContext(nc) as tc:
        with (
            tc.tile_pool(name="cst", bufs=1) as cp,
            tc.tile_pool(name="prm", bufs=1) as pr,
            tc.tile_pool(name="wrk", bufs=9) as wp,
            tc.tile_pool(name="acc", bufs=1, space="PSUM") as ps,
        ):
            inp = cp.tile([128, C_END], f32, tag="inp", name="inp")
            nc.sync.dma_start(inp[:], d_inp[:])
            stim = inp[:, C_STIM:C_STIM + 8]
            gxe = inp[:, C_GXE:C_GXE + 8]
            gye = inp[:, C_GYE:C_GYE + 8]
            pxs = inp[:, C_PXS:C_PXS + 64]
            pys = inp[:, C_PYS:C_PYS + 128]

            def ppc(i):  # patient_params column i as [128, 1]
                return inp[:, C_PP + i:C_PP + i + 1]

            def pt(tag, w=NCHUNK):
                return pr.tile([128, w], f32, tag=tag, name=tag)

            act = nc.scalar.activation
            tt = nc.vector.tensor_tensor
            ts = nc.vector.tensor_scalar
            stt = nc.vector.scalar_tensor_tensor
            rcp = nc.vector.reciprocal

            # ---- per-patient scalars [128, 1] (broadcast on partitions) ----
            th = pt("th", 1)
            ts(th[:], ppc(12), DEG2RAD, None, OP.mult)
            qt = pt("qt", 1)
            tt(qt[:], th[:], th[:], OP.mult)
            ct = pt("ct", 1)          # cos th ~ 1 - th^2/2   (th < 0.0175)
            ts(ct[:], qt[:], -0.5, 1.0, OP.mult, OP.add)
            stp = pt("stp", 1)        # sin th ~ th*(1 - th^2/6)
            ts(stp[:], qt[:], -1.0 / 6.0, 1.0, OP.mult, OP.add)
            st = pt("st", 1)
            tt(st[:], th[:], stp[:], OP.mult)
            dxs = pt("dxs", 1)
            ts(dxs[:], ppc(10), 1.0 / 300.0, None, OP.mult)
            dys = pt("dys", 1)
            ts(dys[:], ppc(11), 1.0 / 300.0, None, OP.mult)
            rho9 = pt("rho9", 1)
            ts(rho9[:], ppc(0), 1.0, 1e-09, OP.mult, OP.add)
            irho = pt("irho", 1)
            rcp(irho[:], rho9[:])

            # ---- electrode rotation [128, 8] ----
            t1 = pt("t1")
            ts(t1[:], gxe, ct[:, 0:1], None, OP.mult)
            t2 = pt("t2")
            stt(t2[:], gye, st[:, 0:1], t1[:], OP.mult, OP.subtract)
            gxn = pt("gxn")           # = -(t2) + dxs
            ts(gxn[:], t2[:], -1.0, dxs[:, 0:1], OP.mult, OP.add)
            t3 = pt("t3")
            ts(t3[:], gxe, st[:, 0:1], None, OP.mult)
            t4 = pt("t4")
            stt(t4[:], gye, ct[:, 0:1], t3[:], OP.mult, OP.add)
            gyn = pt("gyn")
            ts(gyn[:], t4[:], 1.0, dys[:, 0:1], OP.mult, OP.add)

            # ---- exp((gxn + i gyn)/k):  er * (cos + i sin) via DVE polys ----
            ang = pt("ang")
            ts(ang[:], gyn[:], INVK, None, OP.mult)
            qa = pt("qa")
            tt(qa[:], ang[:], ang[:], OP.mult)
            sp = pt("sp")
            ts(sp[:], qa[:], SIN_C[4], SIN_C[3], OP.mult, OP.add)
            for c in (SIN_C[2], SIN_C[1], SIN_C[0]):
                tt(sp[:], sp[:], qa[:], OP.mult)
                ts(sp[:], sp[:], c, None, OP.add)
            si = pt("si")
            tt(si[:], sp[:], ang[:], OP.mult)
            co = pt("co")
            ts(co[:], qa[:], COS_C[4], COS_C[3], OP.mult, OP.add)
            for c in (COS_C[2], COS_C[1], COS_C[0]):
                tt(co[:], co[:], qa[:], OP.mult)
                ts(co[:], co[:], c, None, OP.add)
            er = pt("er")
            act(er[:], gxn[:], AF.Exp, scale=INVK)  # the one ACT table load
            ewr = pt("ewr")
            tt(ewr[:], er[:], co[:], OP.mult)
            ewi = pt("ewi")
            tt(ewi[:], er[:], si[:], OP.mult)

            # ---- z = a*b*(ew - 1)/(b - a*ew)  (complex div) ----
            nr = pt("nr")
            ts(nr[:], ewr[:], AB, -AB, OP.mult, OP.add)
            ni = pt("ni")
            ts(ni[:], ewi[:], AB, None, OP.mult)
            dr = pt("dr")
            ts(dr[:], ewr[:], -A_, B_, OP.mult, OP.add)
            di = pt("di")
            ts(di[:], ewi[:], -A_, None, OP.mult)
            den = pt("den")
            tt(den[:], dr[:], dr[:], OP.mult)
            t5 = pt("t5")
            tt(t5[:], di[:], di[:], OP.mult)
            tt(den[:], den[:], t5[:], OP.add)
            iden = pt("iden")
            rcp(iden[:], den[:])
            q1 = pt("q1")
            tt(q1[:], nr[:], dr[:], OP.mult)
            q2 = pt("q2")
            tt(q2[:], ni[:], di[:], OP.mult)
            tt(q1[:], q1[:], q2[:], OP.add)
            zr = pt("zr")
            tt(zr[:], q1[:], iden[:], OP.mult)
            q3 = pt("q3")
            tt(q3[:], ni[:], dr[:], OP.mult)
            q4 = pt("q4")
            tt(q4[:], nr[:], di[:], OP.mult)
            tt(q3[:], q3[:], q4[:], OP.subtract)
            zi = pt("zi")
            tt(zi[:], q3[:], iden[:], OP.mult)

            # ---- r = |z| and size_base via sqrt(x) = exp(0.5 ln x), packed --
            pk = pt("pk", 16)
            t6 = pt("t6")
            tt(t6[:], zr[:], zr[:], OP.mult)
            t7 = pt("t7")
            tt(t7[:], zi[:], zi[:], OP.mult)
            tt(pk[:, 0:8], t6[:], t7[:], OP.add)          # r^2
            ts(pk[:, 8:16], stim, irho[:, 0:1], 8e-05, OP.mult, OP.mult)
            lnp = pt("lnp", 16)
            act(lnp[:], pk[:], AF.Ln)
            rsb = pt("rsb", 16)
            act(rsb[:], lnp[:], AF.Exp, scale=0.5)
            rr = rsb[:, 0:8]
            sb = rsb[:, 8:16]

            # ---- M, sigma, rs = 1/(sqrt(2) sigma) ----
            rpa = pt("rpa")
            ts(rpa[:], rr, A_, None, OP.add)
            ira = pt("ira")
            rcp(ira[:], rpa[:])
            rpb = pt("rpb")
            ts(rpb[:], rr, B_, None, OP.add)
            irb = pt("irb")
            rcp(irb[:], rpb[:])
            mk = pt("mk")
            tt(mk[:], ira[:], irb[:], OP.subtract)
            me = pt("me")
            ts(me[:], mk[:], K_, 1e-09, OP.mult, OP.add)
            uu = pt("uu")
            rcp(uu[:], me[:])
            vv = pt("vv")
            tt(vv[:], sb, uu[:], OP.mult)
            sg = pt("sg")
            ts(sg[:], vv[:], R2S * DEG2PIX, 0.5, OP.mult, OP.max)
            rsd = pt("rsd")
            ts(rsd[:], sg[:], float(np.sqrt(2.0)), None, OP.mult)
            rs = pt("rs")
            rcp(rs[:], rsd[:])

            # centers scaled for the Square input: -deg2pix * v * rs
            nvx = pt("nvx")
            tt(nvx[:], zr[:], rs[:], OP.mult)
            ts(nvx[:], nvx[:], -DEG2PIX, None, OP.mult)
            nvy = pt("nvy")
            tt(nvy[:], zi[:], rs[:], OP.mult)
            ts(nvy[:], nvy[:], -DEG2PIX, None, OP.mult)

            # ---- Bamp = sigmoid(slp*ie - sh) = 1/(1 + e^sh * exp(-slp*ie)) --
            tie = pt("tie")
            ts(tie[:], stim, 8e-05, -RHEO, OP.mult, OP.add)
            ie = pt("ie")
            ts(ie[:], tie[:], 0.0, None, OP.max)
            exm = pt("exm")
            act(exm[:], ie[:], AF.Exp, scale=-SLP)
            u1 = pt("u1")
            ts(u1[:], exm[:], ESH, 1.0, OP.mult, OP.add)
            bamp = pt("bamp")
            rcp(bamp[:], u1[:])

            # ---- main loop: 8 electrode chunks -> accumulate matmul ----
            # pack the squared distances as [x 0:64 | y 64:192]; one EXP each
            acc = ps.tile([128, 64], f32, tag="acc", name="acc")
            for j in range(NCHUNK):
                jc = slice(j, j + 1)
                sq = wp.tile([128, 192], f32, tag="sq", name="sq")
                dx = wp.tile([128, 64], f32, tag="dx", name="dx")
                ts(dx[:], pxs, rs[:, jc], nvx[:, jc], OP.mult, OP.add)
                tt(sq[:, 0:64], dx[:], dx[:], OP.mult)
                act(sq[:, 64:192], pys, AF.Square, scale=rs[:, jc],
                    bias=nvy[:, jc])
                gp = wp.tile([128, 192], f32, tag="gp", name="gp")
                act(gp[:], sq[:], AF.Exp, scale=-1.0)
                gxb = wp.tile([128, 64], f32, tag="gxb", name="gxb")
                ts(gxb[:], gp[:, 0:64], bamp[:, jc], None, OP.mult)
                nc.tensor.matmul(
                    acc[:], gp[:, 64:192], gxb[:],
                    start=(j == 0), stop=(j == NCHUNK - 1),
                )

            # ---- polynomial + clip (Estrin) on the [128, 64] slice ----
            a0, a1, a2, a3, a4 = (ppc(3 + i) for i in range(5))
            ot = wp.tile([128, 64], f32, tag="ot", name="ot")
            nc.vector.tensor_copy(ot[:], acc[:])
            e1 = wp.tile([128, 64], f32, tag="e1", name="e1")
            ts(e1[:], acc[:], a1, a0, OP.mult, OP.add)
            e2 = wp.tile([128, 64], f32, tag="e2", name="e2")
            ts(e2[:], acc[:], a3, a2, OP.mult, OP.add)
            o2 = wp.tile([128, 64], f32, tag="o2", name="o2")
            tt(o2[:], ot[:], acc[:], OP.mult)
            e3 = wp.tile([128, 64], f32, tag="e3", name="e3")
            stt(e3[:], o2[:], a4, e2[:], OP.mult, OP.add)
            tt(e3[:], o2[:], e3[:], OP.mult)
            tt(e3[:], e3[:], e1[:], OP.add)
            ob = wp.tile([128, 64], f32, tag="ob", name="ob")
            ts(ob[:], e3[:], 0.0, 1.0, OP.max, OP.min)
            nc.sync.dma_start(d_o[:], ob[:])

    nc.finalize()
    _CACHE["nc"] = nc
    return nc


def _prep_in_maps(stim_np: np.ndarray, pp_np: np.ndarray):
    gxe, gye, xs = _host_constants()
    inp_base = np.empty((128, C_END), dtype=np.float32)
    inp_base[:, C_STIM:C_STIM + 8] = (
        stim_np.reshape(-1).astype(np.float32).reshape(NCHUNK, 128).T
    )
    inp_base[:, C_PP:C_PP + 13] = pp_np.reshape(1, 13).astype(np.float32)
    inp_base[:, C_GXE:C_GXE + 8] = gxe
    inp_base[:, C_GYE:C_GYE + 8] = gye
    in_maps = []
    for c in range(N_CORES):
        hh, wq = c // 4, c % 4
        inp = inp_base.copy()
        inp[:, C_PXS:C_PXS + 64] = xs[64 * wq:64 * wq + 64][None, :] * DEG2PIX
        inp[:, C_PYS:C_PYS + 128] = (
            xs[128 * hh:128 * hh + 128][None, :] * DEG2PIX
        )
        in_maps.append({"inp": inp})
    return in_maps


def _assemble(results) -> np.ndarray:
    out = np.empty((OUT, OUT), dtype=np.float32)
    for c in range(N_CORES):
        hh, wq = c // 4, c % 4
        out[128 * hh:128 * hh + 128, 64 * wq:64 * wq + 64] = results[c]["o"]
    return out.reshape(1, 1, OUT, OUT)


def kernel(stimulation: np.ndarray, patient_params: np.ndarray) -> np.ndarray:
    from concourse.bass_utils import run_bass_kernel_spmd

    stim_np = np.asarray(stimulation, dtype=np.float32)
    pp_np = np.asarray(patient_params, dtype=np.float32)
    nc = _build_nc()
    in_maps = _prep_in_maps(stim_np, pp_np)
    res = run_bass_kernel_spmd(nc, in_maps, list(range(N_CORES)))
    return _assemble(res.results)


# revision 19
# speedup vs baseline: 1.3839x; 1.0514x over previous
"""Trainium2 Bass kernel for nn_BioSimulator (phosphene pooling model).

Math: the reference materializes dist2/gauss of shape (1, 1024, 256, 256) and
reduces over the 1024 electrodes.  dist2 is separable:
    dist2[n,h,w] = ((px[w]-vx[n])*s)^2 + ((py[h]-vy[n])*s)^2
so   gauss[n,h,w] = gx[n,w] * gy[n,h]   with
    gx[n,w] = exp(-((px[w]-vx[n])*s*rs_n)^2),  rs_n = 1/(sqrt(2)*sigma_n)
and  out[h,w]  = sum_n Bamp[n] * gy[n,h] * gx[n,w]  — a (H x N) @ (N x W)
matmul with K = 1024.  The per-electrode model configuration (wedge-dipole
retinotopy via complex exp/div, sigma, Bamp) is computed on-chip on [128, 8]
tiles (electrode n = 128*j + p: partition p, chunk column j).

ACT-table discipline: the scalar engine reloads its lookup table (~1.3 us)
whenever the activation function leaves the loaded set, so this kernel only
uses EXP and LN (which share the natural_log_exp_and_others set) plus the
table-free SQUARE.  sin/cos are degree-9/8 polynomials on the vector engine,
sqrt(x) = exp(0.5*ln(x)), and sigmoid = 1/(1 + e^sh * exp(-slope*q)) via DVE
reciprocal.  One table load total.

Sharding: 2x4 grid over the output — core c computes the h-half hh = c // 4
(128 rows) and w-quarter wq = c % 4 (64 cols).  Every core evaluates all 1024
electrodes for its slice (fully local, no collectives); the host stitches the
8 [128, 64] slices into the (1, 1, 256, 256) output.
"""

import numpy as np

GRID = 32
OUT = 256
FOV = 30.0
N_CORES = 8
NCHUNK = 8  # 1024 electrodes / 128 partitions

K_, A_, B_ = 17.3, 0.75, 120.0
SLOPE, HALF, RHEO = 19152642.5, 1.057e-07, 2.39e-05
FREQ, PW, R2S = 300.0, 0.00017, 0.5
DEG2PIX = OUT / (2.0 * FOV)
DEG2RAD = float(np.pi / 180.0)
INVK = 1.0 / K_
AB = A_ * B_
SLP = SLOPE * PW * FREQ          # 976784.7675
ESH = float(np.exp(SLOPE * HALF))  # e^{slope*half}

# sin(x) = x * P(x^2), cos(x) = Q(x^2); least-squares fits on |x| <= 0.9,
# max abs error ~8e-8 in fp32 (used for the gyn/k rotation angle)
SIN_C = (0.9999999999882416, -0.1666666658678421, 0.008333324780098869,
         -0.00019838097971974124, 2.708056858978883e-06)
COS_C = (0.9999999998709687, -0.49999999123379646, 0.041666572790482734,
         -0.0013885406730890894, 2.427793810618373e-05)

# packed input column layout: [stim | pp | gxe | gye | pxs | pys]
C_STIM, C_PP, C_GXE, C_GYE, C_PXS, C_PYS, C_END = 0, 8, 21, 29, 37, 101, 229

_CACHE: dict = {}


def _host_constants():
    """Electrode / pixel grids (input-independent)."""
    if "consts" in _CACHE:
        return _CACHE["consts"]
    xc = np.linspace(-15.0, 15.0, GRID, dtype=np.float32)
    gx, gy = np.meshgrid(xc, xc, indexing="xy")
    # electrode n = 128*j + p  ->  [128, 8] with [p, j] = flat[j*128 + p]
    gxe = gx.reshape(-1).astype(np.float32).reshape(NCHUNK, 128).T.copy()
    gye = gy.reshape(-1).astype(np.float32).reshape(NCHUNK, 128).T.copy()
    xs = np.linspace(-FOV, FOV, OUT, dtype=np.float32)
    _CACHE["consts"] = (gxe, gye, xs)
    return _CACHE["consts"]


def _build_nc():
    """Build the SPMD Bass/Tile program (same program on all 8 cores)."""
    if "nc" in _CACHE:
        return _CACHE["nc"]

    import concourse.bacc as bacc
    import concourse.mybir as mybir
    import concourse.tile as tile

    f32 = mybir.dt.float32
    AF = mybir.ActivationFunctionType
    OP = mybir.AluOpType

    # Bacc (not raw Bass): its compile pipeline runs generate_event_semaphores,
    # which splits multi-sem waits — TRN2 instructions carry at most one wait.
    #
    # Table-set override: the stock insert_act_table_loads maps exp -> the
    # exp_and_others set and ln -> natural_log, which thrashes the ACT table
    # (1.3 us per reload) on our exp/ln/exp sequence.  Putting the combined
    # natural_log_exp_and_others set first makes every activation here (exp,
    # ln, square, copy, relu) resolve to one set -> a single table load.
    # The act_func_set_id is the list INDEX into act_info.json, so the list
    # order must be preserved; instead strip our functions from every other
    # set, leaving natural_log_exp_and_others as the only candidate.
    class _Bacc(bacc.Bacc):
        def insert_act_table_loads(self):
            from concourse.hw_specs import get_activation_tables
            from concourse import bacc as _bacc_mod

            has_activation = any(
                isinstance(i, mybir.InstActivation)
                for b in self.main_func.blocks
                for i in b.instructions
            )
            if not has_activation:
                return
            tabs = get_activation_tables(self.m.arch)
            pref = "natural_log_exp_and_others"
            ours = {
                AF.Exp, AF.Ln, AF.Square, AF.Copy, AF.Relu, AF.Identity,
            }
            tables = [
                (k, (v if k == pref else (v - ours))) for k, v in tabs.items()
            ]
            _bacc_mod._bass_rust.insert_act_table_loads(self, tables)

    nc = _Bacc(None)
    d_inp = nc.declare_dram_parameter("inp", [128, C_END], f32, isOutput=False)
    d_o = nc.declare_dram_parameter("o", [128, 64], f32, isOutput=True)

    with tile.TileContext(nc) as tc:
        with (
            tc.tile_pool(name="cst", bufs=1) as cp,
            tc.tile_pool(name="prm", bufs=1) as pr,
            tc.tile_pool(name="wrk", bufs=9) as wp,
            tc.tile_pool(name="acc", bufs=1, space="PSUM") as ps,
        ):
            inp = cp.tile([128, C_END], f32, tag="inp", name="inp")
            nc.sync.dma_start(inp[:], d_inp[:])
            stim = inp[:, C_STIM:C_STIM + 8]
            gxe = inp[:, C_GXE:C_GXE + 8]
            gye = inp[:, C_GYE:C_GYE + 8]
            pxs = inp[:, C_PXS:C_PXS + 64]
            pys = inp[:, C_PYS:C_PYS + 128]

            def ppc(i):  # patient_params column i as [128, 1]
                return inp[:, C_PP + i:C_PP + i + 1]

            def pt(tag, w=NCHUNK):
                return pr.tile([128, w], f32, tag=tag, name=tag)

            act = nc.scalar.activation
            tt = nc.vector.tensor_tensor
            ts = nc.vector.tensor_scalar
            stt = nc.vector.scalar_tensor_tensor
            rcp = nc.vector.reciprocal

            # ---- per-patient scalars [128, 1] (broadcast on partitions) ----
            th = pt("th", 1)
            ts(th[:], ppc(12), DEG2RAD, None, OP.mult)
            qt = pt("qt", 1)
            tt(qt[:], th[:], th[:], OP.mult)
            ct = pt("ct", 1)          # cos th ~ 1 - th^2/2   (th < 0.0175)
            ts(ct[:], qt[:], -0.5, 1.0, OP.mult, OP.add)
            stp = pt("stp", 1)        # sin th ~ th*(1 - th^2/6)
            ts(stp[:], qt[:], -1.0 / 6.0, 1.0, OP.mult, OP.add)
            st = pt("st", 1)
            tt(st[:], th[:], stp[:], OP.mult)
            dxs = pt("dxs", 1)
            ts(dxs[:], ppc(10), 1.0 / 300.0, None, OP.mult)
            dys = pt("dys", 1)
            ts(dys[:], ppc(11), 1.0 / 300.0, None, OP.mult)
            rho9 = pt("rho9", 1)
            ts(rho9[:], ppc(0), 1.0, 1e-09, OP.mult, OP.add)
            irho = pt("irho", 1)
            rcp(irho[:], rho9[:])

            # ---- electrode rotation [128, 8] ----
            t1 = pt("t1")
            ts(t1[:], gxe, ct[:, 0:1], None, OP.mult)
            t2 = pt("t2")
            stt(t2[:], gye, st[:, 0:1], t1[:], OP.mult, OP.subtract)
            gxn = pt("gxn")           # = -(t2) + dxs
            ts(gxn[:], t2[:], -1.0, dxs[:, 0:1], OP.mult, OP.add)
            t3 = pt("t3")
            ts(t3[:], gxe, st[:, 0:1], None, OP.mult)
            t4 = pt("t4")
            stt(t4[:], gye, ct[:, 0:1], t3[:], OP.mult, OP.add)
            gyn = pt("gyn")
            ts(gyn[:], t4[:], 1.0, dys[:, 0:1], OP.mult, OP.add)

            # ---- exp((gxn + i gyn)/k):  er * (cos + i sin) via DVE polys ----
            ang = pt("ang")
            ts(ang[:], gyn[:], INVK, None, OP.mult)
            qa = pt("qa")
            tt(qa[:], ang[:], ang[:], OP.mult)
            sp = pt("sp")
            ts(sp[:], qa[:], SIN_C[4], SIN_C[3], OP.mult, OP.add)
            for c in (SIN_C[2], SIN_C[1], SIN_C[0]):
                tt(sp[:], sp[:], qa[:], OP.mult)
                ts(sp[:], sp[:], c, None, OP.add)
            si = pt("si")
            tt(si[:], sp[:], ang[:], OP.mult)
            co = pt("co")
            ts(co[:], qa[:], COS_C[4], COS_C[3], OP.mult, OP.add)
            for c in (COS_C[2], COS_C[1], COS_C[0]):
                tt(co[:], co[:], qa[:], OP.mult)
                ts(co[:], co[:], c, None, OP.add)
            er = pt("er")
            act(er[:], gxn[:], AF.Exp, scale=INVK)  # the one ACT table load
            ewr = pt("ewr")
            tt(ewr[:], er[:], co[:], OP.mult)
            ewi = pt("ewi")
            tt(ewi[:], er[:], si[:], OP.mult)

            # ---- z = a*b*(ew - 1)/(b - a*ew)  (complex div) ----
            nr = pt("nr")
            ts(nr[:], ewr[:], AB, -AB, OP.mult, OP.add)
            ni = pt("ni")
            ts(ni[:], ewi[:], AB, None, OP.mult)
            dr = pt("dr")
            ts(dr[:], ewr[:], -A_, B_, OP.mult, OP.add)
            di = pt("di")
            ts(di[:], ewi[:], -A_, None, OP.mult)
            den = pt("den")
            tt(den[:], dr[:], dr[:], OP.mult)
            t5 = pt("t5")
            tt(t5[:], di[:], di[:], OP.mult)
            tt(den[:], den[:], t5[:], OP.add)
            iden = pt("iden")
            rcp(iden[:], den[:])
            q1 = pt("q1")
            tt(q1[:], nr[:], dr[:], OP.mult)
            q2 = pt("q2")
            tt(q2[:], ni[:], di[:], OP.mult)
            tt(q1[:], q1[:], q2[:], OP.add)
            zr = pt("zr")
            tt(zr[:], q1[:], iden[:], OP.mult)
            q3 = pt("q3")
            tt(q3[:], ni[:], dr[:], OP.mult)
            q4 = pt("q4")
            tt(q4[:], nr[:], di[:], OP.mult)
            tt(q3[:], q3[:], q4[:], OP.subtract)
            zi = pt("zi")
            tt(zi[:], q3[:], iden[:], OP.mult)

            # ---- r = |z| and size_base via sqrt(x) = exp(0.5 ln x), packed --
            pk = pt("pk", 16)
            t6 = pt("t6")
            tt(t6[:], zr[:], zr[:], OP.mult)
            t7 = pt("t7")
            tt(t7[:], zi[:], zi[:], OP.mult)
            tt(pk[:, 0:8], t6[:], t7[:], OP.add)          # r^2
            ts(pk[:, 8:16], stim, irho[:, 0:1], 8e-05, OP.mult, OP.mult)
            lnp = pt("lnp", 16)
            act(lnp[:], pk[:], AF.Ln)
            rsb = pt("rsb", 16)
            act(rsb[:], lnp[:], AF.Exp, scale=0.5)
            rr = rsb[:, 0:8]
            sb = rsb[:, 8:16]

            # ---- M, sigma, rs = 1/(sqrt(2) sigma) ----
            rpa = pt("rpa")
            ts(rpa[:], rr, A_, None, OP.add)
            ira = pt("ira")
            rcp(ira[:], rpa[:])
            rpb = pt("rpb")
            ts(rpb[:], rr, B_, None, OP.add)
            irb = pt("irb")
            rcp(irb[:], rpb[:])
            mk = pt("mk")
            tt(mk[:], ira[:], irb[:], OP.subtract)
            me = pt("me")
            ts(me[:], mk[:], K_, 1e-09, OP.mult, OP.add)
            uu = pt("uu")
            rcp(uu[:], me[:])
            vv = pt("vv")
            tt(vv[:], sb, uu[:], OP.mult)
            sg = pt("sg")
            ts(sg[:], vv[:], R2S * DEG2PIX, 0.5, OP.mult, OP.max)
            rsd = pt("rsd")
            ts(rsd[:], sg[:], float(np.sqrt(2.0)), None, OP.mult)
            rs = pt("rs")
            rcp(rs[:], rsd[:])

            # centers scaled for the Square input: -deg2pix * v * rs
            nvx = pt("nvx")
            tt(nvx[:], zr[:], rs[:], OP.mult)
            ts(nvx[:], nvx[:], -DEG2PIX, None, OP.mult)
            nvy = pt("nvy")
            tt(nvy[:], zi[:], rs[:], OP.mult)
            ts(nvy[:], nvy[:], -DEG2PIX, None, OP.mult)

            # ---- Bamp = sigmoid(slp*ie - sh) = 1/(1 + e^sh * exp(-slp*ie)) --
            tie = pt("tie")
            ts(tie[:], stim, 8e-05, -RHEO, OP.mult, OP.add)
            ie = pt("ie")
            ts(ie[:], tie[:], 0.0, None, OP.max)
            exm = pt("exm")
            act(exm[:], ie[:], AF.Exp, scale=-SLP)
            u1 = pt("u1")
            ts(u1[:], exm[:], ESH, 1.0, OP.mult, OP.add)
            bamp = pt("bamp")
            rcp(bamp[:], u1[:])

            # ---- main loop: 8 electrode chunks -> accumulate matmul ----
            # pack the squared distances as [x 0:64 | y 64:192]; one EXP each
            acc = ps.tile([128, 64], f32, tag="acc", name="acc")
            for j in range(NCHUNK):
                jc = slice(j, j + 1)
                sq = wp.tile([128, 192], f32, tag="sq", name="sq")
                dx = wp.tile([128, 64], f32, tag="dx", name="dx")
                ts(dx[:], pxs, rs[:, jc], nvx[:, jc], OP.mult, OP.add)
                tt(sq[:, 0:64], dx[:], dx[:], OP.mult)
                act(sq[:, 64:192], pys, AF.Square, scale=rs[:, jc],
                    bias=nvy[:, jc])
                gp = wp.tile([128, 192], f32, tag="gp", name="gp")
                act(gp[:], sq[:], AF.Exp, scale=-1.0)
                gxb = wp.tile([128, 64], f32, tag="gxb", name="gxb")
                ts(gxb[:], gp[:, 0:64], bamp[:, jc], None, OP.mult)
                nc.tensor.matmul(
                    acc[:], gp[:, 64:192], gxb[:],
                    start=(j == 0), stop=(j == NCHUNK - 1),
                )

            # ---- polynomial + clip (Estrin) on the [128, 64] slice ----
            a0, a1, a2, a3, a4 = (ppc(3 + i) for i in range(5))
            ot = wp.tile([128, 64], f32, tag="ot", name="ot")
            nc.vector.tensor_copy(ot[:], acc[:])
            e1 = wp.tile([128, 64], f32, tag="e1", name="e1")
            ts(e1[:], acc[:], a1, a0, OP.mult, OP.add)
            e2 = wp.tile([128, 64], f32, tag="e2", name="e2")
            ts(e2[:], acc[:], a3, a2, OP.mult, OP.add)
            o2 = wp.tile([128, 64], f32, tag="o2", name="o2")
            tt(o2[:], ot[:], acc[:], OP.mult)
            e3 = wp.tile([128, 64], f32, tag="e3", name="e3")
            stt(e3[:], o2[:], a4, e2[:], OP.mult, OP.add)
            tt(e3[:], o2[:], e3[:], OP.mult)
            tt(e3[:], e3[:], e1[:], OP.add)
            ob = wp.tile([128, 64], f32, tag="ob", name="ob")
            ts(ob[:], e3[:], 0.0, 1.0, OP.max, OP.min)
            nc.sync.dma_start(d_o[:], ob[:])

    nc.finalize()
    _CACHE["nc"] = nc
    return nc


def _prep_in_maps(stim_np: np.ndarray, pp_np: np.ndarray):
    gxe, gye, xs = _host_constants()
    inp_base = np.empty((128, C_END), dtype=np.float32)
    inp_base[:, C_STIM:C_STIM + 8] = (
        stim_np.reshape(-1).astype(np.float32).reshape(NCHUNK, 128).T
    )
    inp_base[:, C_PP:C_PP + 13] = pp_np.reshape(1, 13).astype(np.float32)
    inp_base[:, C_GXE:C_GXE + 8] = gxe
    inp_base[:, C_GYE:C_GYE + 8] = gye
    in_maps = []
    for c in range(N_CORES):
        hh, wq = c // 4, c % 4
        inp = inp_base.copy()
        inp[:, C_PXS:C_PXS + 64] = xs[64 * wq:64 * wq + 64][None, :] * DEG2PIX
        inp[:, C_PYS:C_PYS + 128] = (
            xs[128 * hh:128 * hh + 128][None, :] * DEG2PIX
        )
        in_maps.append({"inp": inp})
    return in_maps


def _assemble(results) -> np.ndarray:
    out = np.empty((OUT, OUT), dtype=np.float32)
    for c in range(N_CORES):
        hh, wq = c // 4, c % 4
        out[128 * hh:128 * hh + 128, 64 * wq:64 * wq + 64] = results[c]["o"]
    return out.reshape(1, 1, OUT, OUT)


def kernel(stimulation: np.ndarray, patient_params: np.ndarray) -> np.ndarray:
    from concourse.bass_utils import run_bass_kernel_spmd

    stim_np = np.asarray(stimulation, dtype=np.float32)
    pp_np = np.asarray(patient_params, dtype=np.float32)
    nc = _build_nc()
    in_maps = _prep_in_maps(stim_np, pp_np)
    res = run_bass_kernel_spmd(nc, in_maps, list(range(N_CORES)))
    return _assemble(res.results)
